# revision 19
# baseline (speedup 1.0000x reference)
"""CrossAttention Trainium2 kernel (8 NeuronCores, SPMD), bf16 compute.

Sharding: data-parallel over batch B=2, tensor-parallel over the 16 heads in
4 groups of 4 heads -> 8 cores, one (batch, head-group) pair each. Each core
computes its 4 heads' Q/K/V projections, masked softmax cross-attention, and
its partial output projection y_g = softmax(q k^T * scale) v @ Wo[:, g].T.
The host sums the 4 partial outputs per batch (the Wo row-split all-reduce,
done at unshard time) and adds the v-bias term Wo @ b_v, which is constant
across rows and factors out of the attention (softmax rows sum to 1).

Numerics: inputs are cast to bf16 on the host; every matmul runs bf16 x bf16
with fp32 PSUM accumulation; softmax statistics (denominator, reciprocal,
normalization) stay fp32. The partial output returns as bf16; the host
reduces in fp32.

Layout: the PE contracts over the partition dim, so activations and weights
arrive contraction-major, pre-swizzled on the host into the exact SBUF tile
byte layout so every device load is one large contiguous DMA (the HWDGE
rings pay ~1.5us per DMA instruction; strided many-line DMAs are slow).
Attention is computed scores-transposed: ST[m, n] per head, so the PV matmul
contracts over m directly. The softmax denominator comes from a prepended
ones-column on the v stationary operand (row 0 of the PV psum), where the
reciprocal can read it at partition 0 without a shuffle. exp() is
unnormalized (scores*scale are bounded); mask zeros apply multiplicatively
after exp. The two heads of a pair run as concurrent row-tiled matmuls
(contraction 64 each, auto tile_position via base partition 0/64).

Schedule (single pass, DMA-overlapped):
  prologue: weights/x/ctx stream in as 10 large DMAs over 3 rings, ordered
            so qproj starts at ~t3 and the first scores fire at ~t8.
  stage 1:  scores+exp+mask heads 0,1; V projection and the remaining
            K/Q projections fill PE slack between score pairs.
  stage 2:  PV accumulation heads 0,1 interleaved with scores heads 2,3
            (ACT-paced); K projection et1 rides along. Heads 0,1 are
            normalized out of PSUM at the stage boundary.
  stage 3:  PV heads 2,3 split by n-half; normalization (PSUM reciprocal +
            PE ones-broadcast) and the output projection + stores pipeline
            against the second half's PV.
"""

import numpy as np
import ml_dtypes

import concourse.bass as bass
import concourse.bacc as bacc
import concourse.mybir as mybir
import concourse.tile as tile
from concourse.bass_utils import run_bass_kernel_spmd

DIM = 1024
HEAD_DIM = 64
NUM_HEADS = 16
SCALE = HEAD_DIM**-0.5
B, N, M = 2, 1024, 2048
HPC = 4  # heads per core
E = HPC * HEAD_DIM  # 256: per-core projection width
P = 128
F32 = mybir.dt.float32
BF16 = mybir.dt.bfloat16
CT = DIM // P  # 8 contraction tiles
MT = M // P  # 16 m tiles
CHM = M // 512  # 4 ctx column chunks
NB = N // P  # 8 output row blocks
OC = DIM // 512  # 2 output column chunks


def build_program():
    nc = bacc.Bacc("TRN2", target_bir_lowering=False, debug=False, num_devices=8)

    # host pre-swizzles everything into SBUF tile layout (see make_in_maps)
    xT_d = nc.dram_tensor("xT", [P, CT * N], BF16, kind="ExternalInput").ap()
    ctxT_d = nc.dram_tensor("ctxT", [P, CHM * CT * 512], BF16, kind="ExternalInput").ap()
    maskt_d = nc.dram_tensor("maskt", [M, N], BF16, kind="ExternalInput").ap()
    wqT_d = nc.dram_tensor("wqT", [P, CT * E], BF16, kind="ExternalInput").ap()
    wkT_d = nc.dram_tensor("wkT", [P, CT * E], BF16, kind="ExternalInput").ap()
    wvT_d = nc.dram_tensor("wvT", [P, CT * E], BF16, kind="ExternalInput").ap()
    woT_d = nc.dram_tensor("woT", [P, (E // P) * DIM], BF16, kind="ExternalInput").ap()
    bk_d = nc.dram_tensor("bk", [E], F32, kind="ExternalInput").ap()
    y_d = nc.dram_tensor("y", [NB, OC, P, 512], BF16, kind="ExternalOutput").ap()
    import os

    kdbg = bool(os.environ.get("KDBG"))
    if kdbg:
        dbg_d = nc.dram_tensor("dbg", [4, 512], F32, kind="ExternalOutput").ap()

    Exp = mybir.ActivationFunctionType.Exp
    Copy = mybir.ActivationFunctionType.Copy

    from contextlib import ExitStack

    with tile.TileContext(nc) as tc, ExitStack() as ctx:
        const = ctx.enter_context(tc.tile_pool(name="const", bufs=1))
        bk_sb = const.tile([P, E // P], F32)
        ones64 = const.tile([P, HEAD_DIM], F32)

        persist = ctx.enter_context(tc.tile_pool(name="persist", bufs=1))
        qT = persist.tile([P, E // P, N], BF16)
        kT = persist.tile([P, E // P, M], BF16)
        # cols 0:64 = V, col 64 = ones (denominator lands on psum row 64)
        vaug = persist.tile([P, MT, HPC, HEAD_DIM + 1], BF16)
        woT = persist.tile([P, E // P, DIM], BF16)
        otn2 = persist.tile([P, E // P, N], BF16)

        bwork = ctx.enter_context(tc.tile_pool(name="bwork", bufs=4))
        maskp = ctx.enter_context(tc.tile_pool(name="maskp", bufs=6))
        rbp = ctx.enter_context(tc.tile_pool(name="rbp", bufs=2))

        exmp = ctx.enter_context(tc.tile_pool(name="exmp", bufs=1))
        # masked exp(scores) parked per m-tile; one buffer reused across
        # head pairs (WAR: stage-2 rewrites a tile only after its PV read)
        exmst = exmp.tile([P, MT, 2, N], BF16)

        mask_tiles = {}
        mask_rings = [nc.sync, nc.gpsimd, nc.scalar]

        def load_masks(upto):
            """Issue mask DMAs up to tile index `upto`; first three ride the
            gpsimd SWDGE (the HWDGE rings are busy with weights/x/ctx), the
            rest rotate over all three rings. The 6-buffer pool provides
            back-pressure against loading too far ahead."""
            while len(mask_tiles) < min(upto, 2 * MT):
                i = len(mask_tiles)
                mt = i % MT
                mk = maskp.tile([P, N], BF16, tag="mk", name="mk")
                ring = nc.gpsimd if i < 3 else mask_rings[i % 3]
                ring.dma_start(out=mk, in_=maskt_d[mt * P : (mt + 1) * P, :])
                mask_tiles[i] = mk

        def scores_half(spool, sbufs, hp, mt, chn):
            """score pair for head pair hp at (m-tile, n-half); the two
            heads run as concurrent row-tiled matmuls."""
            st = spool.tile([P, 2, 512], F32, tag="st", name="st", bufs=sbufs)
            for hl in range(2):
                erow = slice(hl * HEAD_DIM, (hl + 1) * HEAD_DIM)
                nc.tensor.matmul(
                    st[:, hl, :],
                    lhsT=kT[erow, hp, mt * P : (mt + 1) * P],
                    rhs=qT[erow, hp, chn * 512 : (chn + 1) * 512],
                    start=True,
                    stop=True,
                )
            return st

        def expmask_half(st, mt, chn, mk):
            ex = bwork.tile([P, 2, 512], BF16, tag="ex", name="ex")
            nc.scalar.activation(ex, st, Exp, scale=float(SCALE))
            mks = mk[:, chn * 512 : (chn + 1) * 512]
            mkc = bass.AP(mks.tensor, mks.offset, [mks.ap[0], [0, 2], mks.ap[1]])
            nc.vector.tensor_mul(
                exmst[:, mt, :, chn * 512 : (chn + 1) * 512], ex, mkc
            )

        def emit_pv(ot_ps, hp, mt):
            for hl in range(2):
                h = hp * 2 + hl
                for chn in range(2):
                    nc.tensor.matmul(
                        ot_ps[hl * 2 + chn],
                        lhsT=vaug[:, mt, h, :],
                        rhs=exmst[:, mt, hl, chn * 512 : (chn + 1) * 512],
                        start=(mt == 0),
                        stop=(mt == MT - 1),
                    )

        dbg_last = []

        def norm_recip(ot_ps, rbps):
            """stage the denominator row (PSUM partition 64) through SBUF,
            then a K=1 matmul at tile row-position 64 broadcasts it to PSUM
            rows 0:64. (reciprocal_approx_fast only works on SBUF inputs at
            partition base 0 on HW, so the reciprocal happens after.)"""
            dn1 = rbp.tile([P, 512], F32, tag="dn1", name="dn1")
            nc.vector.tensor_copy(
                dn1[HEAD_DIM : HEAD_DIM + 1, :],
                ot_ps[HEAD_DIM : HEAD_DIM + 1, :],
            )
            dbg_last.clear()
            dbg_last.append(dn1)
            dp = rbps.tile([HEAD_DIM, 512], F32, tag="rp", name="rp")
            nc.tensor.matmul(
                dp,
                lhsT=ones64[HEAD_DIM : HEAD_DIM + 1, :],
                rhs=dn1[HEAD_DIM : HEAD_DIM + 1, :],
                start=True,
                stop=True,
            )
            return dp

        def norm_apply(ot_ps, dp, h, chn):
            """reciprocal of the broadcast denominator (SBUF, base 0), then
            multiply rows 0:64 of the PV psum into the head's otn2 slot
            (odd heads shift partitions via DMA)."""
            et, hl = divmod(h, 2)
            sl = slice(chn * 512, (chn + 1) * 512)
            dd = rbp.tile([HEAD_DIM, 512], F32, tag="dd", name="dd")
            nc.vector.tensor_copy(dd, dp)
            rb = rbp.tile([HEAD_DIM, 512], F32, tag="rb", name="rb")
            nc.vector.reciprocal_approx_fast(out=rb, in_=dd)
            if hl == 0:
                nc.vector.tensor_mul(
                    otn2[:HEAD_DIM, et, sl], ot_ps[:HEAD_DIM, :], rb
                )
            else:
                tmp = rbp.tile([HEAD_DIM, 512], BF16, tag="tmp", name="tmp")
                nc.vector.tensor_mul(tmp, ot_ps[:HEAD_DIM, :], rb)
                # partition shift 0:64 -> 64:128 via SBUF-SBUF DMA
                nc.gpsimd.dma_start(out=otn2[HEAD_DIM:P, et, sl], in_=tmp)

        with tc.tile_pool(name="wctx", bufs=1) as wctx_pool:
            wkT = wctx_pool.tile([P, CT, E], BF16)
            wvT = wctx_pool.tile([P, CT, E], BF16)
            ctxT = wctx_pool.tile([P, CHM, CT, 512], BF16)

            with (
                tc.tile_pool(name="qx", bufs=1) as qx_pool,
                tc.tile_pool(name="ppsA", bufs=1, space="PSUM") as ppsA,
                tc.tile_pool(name="vps", bufs=1, space="PSUM") as vps,
                tc.tile_pool(name="kps", bufs=2, space="PSUM") as kps,
            ):
                wqT = qx_pool.tile([P, CT, E], BF16)
                xT = qx_pool.tile([P, CT, N], BF16)

                # ---------- input DMA: large contiguous transfers, ordered
                # per ring for earliest compute start (critical chain:
                # wq+x -> qproj; wk+ctx chunk0 -> kproj -> first scores) ----
                # sync ring: wq, then x in two halves
                nc.sync.dma_start(out=wqT, in_=wqT_d)
                nc.sync.dma_start(out=xT[:, 0:4, :], in_=xT_d[:, 0 : 4 * N])
                # gpsimd SWDGE: bias, x second half, first masks, wv
                nc.gpsimd.dma_start(out=bk_sb, in_=bk_d.rearrange("(t p) -> p t", p=P))
                nc.gpsimd.dma_start(out=xT[:, 4:8, :], in_=xT_d[:, 4 * N : 8 * N])
                load_masks(3)
                nc.gpsimd.dma_start(out=wvT, in_=wvT_d)
                # scalar ring: wk, ctx in four 1MB chunks, wo
                nc.scalar.dma_start(out=wkT, in_=wkT_d)
                csz = CT * 512
                for c in range(CHM):
                    nc.scalar.dma_start(
                        out=ctxT[:, c, :, :], in_=ctxT_d[:, c * csz : (c + 1) * csz]
                    )
                nc.scalar.dma_start(out=woT, in_=woT_d)

                def emit_qproj(et, chn):
                    pq = ppsA.tile([P, 512], F32, tag="pq", name="pq")
                    for j in range(CT):
                        nc.tensor.matmul(
                            pq,
                            lhsT=wqT[:, j, et * P : (et + 1) * P],
                            rhs=xT[:, j, chn * 512 : (chn + 1) * 512],
                            start=(j == 0),
                            stop=(j == CT - 1),
                        )
                    nc.vector.tensor_copy(
                        qT[:, et, chn * 512 : (chn + 1) * 512], pq
                    )

                def emit_kproj(et, chm):
                    pk = kps.tile([P, 512], F32, tag="pk", name="pk")
                    for j in range(CT):
                        nc.tensor.matmul(
                            pk,
                            lhsT=wkT[:, j, et * P : (et + 1) * P],
                            rhs=ctxT[:, chm, j, :],
                            start=(j == 0),
                            stop=(j == CT - 1),
                        )
                    nc.vector.tensor_scalar_add(
                        kT[:, et, chm * 512 : (chm + 1) * 512],
                        pk,
                        bk_sb[:, et : et + 1],
                    )

                def emit_vproj(mt):
                    pv = vps.tile([P, HPC, HEAD_DIM], F32, tag="pv", name="pv")
                    for j in range(CT):
                        nc.tensor.matmul(
                            pv,
                            lhsT=ctxT[:, mt // 4, j, (mt % 4) * P : (mt % 4 + 1) * P],
                            rhs=wvT[:, j, :],
                            start=(j == 0),
                            stop=(j == CT - 1),
                        )
                    nc.vector.tensor_copy(vaug[:, mt, :, :HEAD_DIM], pv)

                # ---------- prologue: qproj et0, first kproj ----------
                # ones column only; v evictions fill cols 0:64
                nc.vector.memset(vaug[:, :, :, HEAD_DIM : HEAD_DIM + 1], 1.0)
                nc.vector.memset(ones64, 1.0)
                emit_qproj(0, 0)
                emit_qproj(0, 1)
                emit_kproj(0, 0)

                # ---------- stage 1: scores heads 0,1 + projections ----------
                # kproj/qproj leftovers keyed to ctx chunk arrival; vproj
                # lags two slots behind its ctx chunk (wv lands ~t8)
                s1_extras = {
                    0: lambda: emit_qproj(1, 0),
                    1: lambda: emit_qproj(1, 1),
                    2: lambda: emit_kproj(0, 1),
                    4: lambda: emit_kproj(0, 2),
                    6: lambda: emit_kproj(0, 3),
                }
                with tc.tile_pool(name="sps1", bufs=1, space="PSUM") as sps1:
                    for mt in range(MT):
                        load_masks(mt + 4)
                        mk = mask_tiles[mt]
                        st0 = scores_half(sps1, 2, 0, mt, 0)
                        st1 = scores_half(sps1, 2, 0, mt, 1)
                        expmask_half(st0, mt, 0, mk)
                        expmask_half(st1, mt, 1, mk)
                        if mt >= 2:
                            emit_vproj(mt - 2)
                        if mt in s1_extras:
                            s1_extras[mt]()
                    emit_vproj(MT - 2)
                    emit_vproj(MT - 1)

            # ---------- stage 2: PV heads 0,1 + scores heads 2,3 ----------
            s2_kproj = {0: 0, 2: 1, 5: 2, 8: 3}
            with (
                tc.tile_pool(name="ops0", bufs=1, space="PSUM") as ops0,
                tc.tile_pool(name="sps2", bufs=1, space="PSUM") as sps2,
                tc.tile_pool(name="kps2", bufs=1, space="PSUM") as kps2,
                tc.tile_pool(name="rbps", bufs=1, space="PSUM") as rbps,
            ):
                def emit_kproj2(chm):
                    pk = kps2.tile([P, 512], F32, tag="pk2", name="pk2")
                    for j in range(CT):
                        nc.tensor.matmul(
                            pk,
                            lhsT=wkT[:, j, P : 2 * P],
                            rhs=ctxT[:, chm, j, :],
                            start=(j == 0),
                            stop=(j == CT - 1),
                        )
                    nc.vector.tensor_scalar_add(
                        kT[:, 1, chm * 512 : (chm + 1) * 512],
                        pk,
                        bk_sb[:, 1:2],
                    )

                ot_ps0 = [
                    ops0.tile([HEAD_DIM + 1, 512], F32, tag=f"o{i}", name=f"o{i}")
                    for i in range(4)
                ]
                for mt in range(MT):
                    load_masks(MT + mt + 4)
                    mk = mask_tiles[MT + mt]
                    if mt in s2_kproj:
                        emit_kproj2(s2_kproj[mt])
                    st0 = scores_half(sps2, 1, 1, mt, 0)
                    emit_pv(ot_ps0, 0, mt)
                    expmask_half(st0, mt, 0, mk)
                    st1 = scores_half(sps2, 1, 1, mt, 1)
                    expmask_half(st1, mt, 1, mk)
                # normalize heads 0,1 straight out of PSUM
                for hl in range(2):
                    for chn in range(2):
                        ps = ot_ps0[hl * 2 + chn]
                        rp = norm_recip(ps, rbps)
                        if kdbg and hl == 0 and chn == 0:
                            dbg_sb = rbp.tile([P, 512], F32, tag="dbg", name="dbg")
                            nc.vector.tensor_copy(dbg_sb[0:2, :], rp[0:2, :])
                            nc.sync.dma_start(
                                out=dbg_d[0:1, :],
                                in_=dbg_last[0][HEAD_DIM : HEAD_DIM + 1, :],
                            )
                            nc.sync.dma_start(out=dbg_d[2:4, :], in_=dbg_sb[0:2, :])
                        norm_apply(ps, rp, hl, chn)

        # ---------- stage 3: PV heads 2,3 (split by n-half) + output ----------
        with (
            tc.tile_pool(name="ops1", bufs=1, space="PSUM") as ops1,
            tc.tile_pool(name="rbps1", bufs=2, space="PSUM") as rbps1,
            tc.tile_pool(name="ypsum", bufs=2, space="PSUM") as ypsum,
            tc.tile_pool(name="ypool", bufs=4) as ypool,
        ):
            ot_ps1 = {
                (hl, chn): ops1.tile(
                    [HEAD_DIM + 1, 512], F32, tag=f"p{hl}{chn}", name=f"p{hl}{chn}"
                )
                for hl in range(2)
                for chn in range(2)
            }
            rings = [nc.sync, nc.scalar, nc.gpsimd]
            evict_eng = [nc.scalar, nc.vector]

            def outproj_quarter(c):
                for nb in range(4 * c, 4 * c + 4):
                    for oc in range(OC):
                        i = nb * OC + oc
                        yp = ypsum.tile([P, 512], F32, tag="yp", name="yp")
                        for et in range(E // P):
                            nc.tensor.matmul(
                                yp,
                                lhsT=otn2[:, et, nb * P : (nb + 1) * P],
                                rhs=woT[:, et, oc * 512 : (oc + 1) * 512],
                                start=(et == 0),
                                stop=(et == E // P - 1),
                            )
                        ys = ypool.tile([P, 512], BF16, tag="ys", name="ys")
                        eng = evict_eng[i % 2]
                        if eng is nc.scalar:
                            nc.scalar.activation(ys, yp, Copy)
                        else:
                            nc.vector.tensor_copy(ys, yp)
                        rings[i % 3].dma_start(out=y_d[nb, oc], in_=ys)

            def pv_chain(hl, chn):
                for mt in range(MT):
                    nc.tensor.matmul(
                        ot_ps1[(hl, chn)],
                        lhsT=vaug[:, mt, 2 + hl, :],
                        rhs=exmst[:, mt, hl, chn * 512 : (chn + 1) * 512],
                        start=(mt == 0),
                        stop=(mt == MT - 1),
                    )

            # n-half 0: PV both heads, then normalize (overlaps n-half 1 PV)
            pv_chain(0, 0)
            pv_chain(1, 0)
            for hl in range(2):
                ps = ot_ps1[(hl, 0)]
                rp = norm_recip(ps, rbps1)
                norm_apply(ps, rp, 2 + hl, 0)
            # n-half 1: PV both heads
            pv_chain(0, 1)
            pv_chain(1, 1)
            # reciprocal first (DVE) so outproj q0 keeps the PE busy while
            # the rest of the c1 normalization completes
            rps = []
            for hl in range(2):
                rps.append(norm_recip(ot_ps1[(hl, 1)], rbps1))
            outproj_quarter(0)
            for hl in range(2):
                norm_apply(ot_ps1[(hl, 1)], rps[hl], 2 + hl, 1)
            outproj_quarter(1)

    nc.compile()
    return nc


_NC_CACHE = []


def _get_nc():
    if not _NC_CACHE:
        _NC_CACHE.append(build_program())
    return _NC_CACHE[0]


def _pack(a, p=P):
    """[(j p), f] -> [p, j*f] contiguous (SBUF tile byte layout)."""
    j = a.shape[0] // p
    return np.ascontiguousarray(
        a.reshape(j, p, a.shape[1]).transpose(1, 0, 2).reshape(p, -1)
    )


def make_in_maps(x, context, mask, Wq, Wkv, b_kv, Wo):
    bf = ml_dtypes.bfloat16
    x = np.asarray(x, dtype=np.float32)
    context = np.asarray(context, dtype=np.float32)
    mask = np.asarray(mask)
    Wq = np.asarray(Wq, dtype=np.float32)
    Wkv = np.asarray(Wkv, dtype=np.float32)
    b_kv = np.asarray(b_kv, dtype=np.float32)
    Wo = np.asarray(Wo, dtype=np.float32)

    in_maps = []
    for b in range(B):
        xtb = _pack(x[b].T.astype(bf))
        # ctx swizzled chunk-major: [p, chm, j, 512]
        ctb = np.ascontiguousarray(
            context[b]
            .T.astype(bf)
            .reshape(CT, P, CHM, 512)
            .transpose(1, 2, 0, 3)
            .reshape(P, -1)
        )
        mtb = np.ascontiguousarray(mask[b].T).astype(bf)
        for g in range(NUM_HEADS // HPC):
            sl = slice(E * g, E * (g + 1))
            in_maps.append(
                {
                    "xT": xtb,
                    "ctxT": ctb,
                    "maskt": mtb,
                    "wqT": _pack(np.ascontiguousarray(Wq[sl].T).astype(bf)),
                    "wkT": _pack(np.ascontiguousarray(Wkv[sl].T).astype(bf)),
                    "wvT": _pack(
                        np.ascontiguousarray(
                            Wkv[DIM + E * g : DIM + E * (g + 1)].T
                        ).astype(bf)
                    ),
                    "woT": _pack(np.ascontiguousarray(Wo[:, sl].T).astype(bf)),
                    "bk": np.ascontiguousarray(b_kv[sl]),
                }
            )
    return in_maps


def combine_outputs(ys, b_kv, Wo):
    """ys: list of 8 per-core partial outputs [NB, OC, P, 512] (packed),
    core order (b, g)."""
    b_v = np.asarray(b_kv, dtype=np.float32)[DIM:]
    ybias = np.asarray(Wo, dtype=np.float32) @ b_v  # [DIM]
    out = np.empty((B, N, DIM), dtype=np.float32)
    G = NUM_HEADS // HPC
    for b in range(B):
        acc = np.asarray(ys[G * b], dtype=np.float32)
        for g in range(1, G):
            acc = acc + np.asarray(ys[G * b + g], dtype=np.float32)
        out[b] = acc.transpose(0, 2, 1, 3).reshape(N, DIM) + ybias[None, :]
    return out


def kernel(x, context, mask, Wq, Wkv, b_kv, Wo):
    nc = _get_nc()
    in_maps = make_in_maps(x, context, mask, Wq, Wkv, b_kv, Wo)
    res = run_bass_kernel_spmd(nc, in_maps, core_ids=list(range(8)))
    ys = [m["y"] for m in res.results]
    return combine_outputs(ys, b_kv, Wo)


# revision 29
# speedup vs baseline: 1.0469x; 1.0469x over previous
"""CrossAttention Trainium2 kernel (8 NeuronCores, SPMD), bf16 compute.

Sharding: data-parallel over batch B=2, tensor-parallel over the 16 heads in
4 groups of 4 heads -> 8 cores, one (batch, head-group) pair each. Each core
computes its 4 heads' Q/K/V projections, masked softmax cross-attention, and
its partial output projection y_g = softmax(q k^T * scale) v @ Wo[:, g].T.
The host sums the 4 partial outputs per batch (the Wo row-split all-reduce,
done at unshard time) and adds the v-bias term Wo @ b_v, which is constant
across rows and factors out of the attention (softmax rows sum to 1).

Numerics: inputs are cast to bf16 on the host; every matmul runs bf16 x bf16
with fp32 PSUM accumulation; softmax statistics (denominator, reciprocal,
normalization) stay fp32. The partial output returns as bf16; the host
reduces in fp32.

Layout: the PE contracts over the partition dim, so activations and weights
arrive contraction-major, pre-swizzled on the host into the exact SBUF tile
byte layout so every device load is one large contiguous DMA (the HWDGE
rings pay ~1.5us per DMA instruction; strided many-line DMAs are slow).
Attention is computed scores-transposed: ST[m, n] per head, so the PV matmul
contracts over m directly. The softmax denominator comes from a prepended
ones-column on the v stationary operand (row 0 of the PV psum), where the
reciprocal can read it at partition 0 without a shuffle. exp() is
unnormalized (scores*scale are bounded); mask zeros apply multiplicatively
after exp. The two heads of a pair run as concurrent row-tiled matmuls
(contraction 64 each, auto tile_position via base partition 0/64).

Schedule (single pass, DMA-overlapped):
  prologue: weights/x/ctx stream in as 10 large DMAs over 3 rings, ordered
            so qproj starts at ~t3 and the first scores fire at ~t8.
  stage 1:  scores+exp+mask heads 0,1; V projection and the remaining
            K/Q projections fill PE slack between score pairs.
  stage 2:  PV accumulation heads 0,1 interleaved with scores heads 2,3
            (ACT-paced); K projection et1 rides along. Heads 0,1 are
            normalized out of PSUM at the stage boundary.
  stage 3:  PV heads 2,3 split by n-half; normalization (PSUM reciprocal +
            PE ones-broadcast) and the output projection + stores pipeline
            against the second half's PV.
"""

import numpy as np
import ml_dtypes

import concourse.bass as bass
import concourse.bacc as bacc
import concourse.mybir as mybir
import concourse.tile as tile
from concourse.bass_utils import run_bass_kernel_spmd

DIM = 1024
HEAD_DIM = 64
NUM_HEADS = 16
SCALE = HEAD_DIM**-0.5
B, N, M = 2, 1024, 2048
HPC = 4  # heads per core
E = HPC * HEAD_DIM  # 256: per-core projection width
P = 128
F32 = mybir.dt.float32
BF16 = mybir.dt.bfloat16
CT = DIM // P  # 8 contraction tiles
MT = M // P  # 16 m tiles
CHM = M // 512  # 4 ctx column chunks
NB = N // P  # 8 output row blocks
OC = DIM // 512  # 2 output column chunks


def build_program():
    nc = bacc.Bacc("TRN2", target_bir_lowering=False, debug=False, num_devices=8)

    # host pre-swizzles everything into SBUF tile layout (see make_in_maps)
    xT_d = nc.dram_tensor("xT", [P, CT * N], BF16, kind="ExternalInput").ap()
    ctxT_d = nc.dram_tensor("ctxT", [P, CHM * CT * 512], BF16, kind="ExternalInput").ap()
    maskt_d = nc.dram_tensor("maskt", [M, N], BF16, kind="ExternalInput").ap()
    wqT_d = nc.dram_tensor("wqT", [P, CT * E], BF16, kind="ExternalInput").ap()
    wkT_d = nc.dram_tensor("wkT", [P, CT * E], BF16, kind="ExternalInput").ap()
    wvT_d = nc.dram_tensor("wvT", [P, CT * E], BF16, kind="ExternalInput").ap()
    woT_d = nc.dram_tensor("woT", [P, (E // P) * DIM], BF16, kind="ExternalInput").ap()
    bk_d = nc.dram_tensor("bk", [E], F32, kind="ExternalInput").ap()
    y_d = nc.dram_tensor("y", [NB, OC, P, 512], BF16, kind="ExternalOutput").ap()

    Exp = mybir.ActivationFunctionType.Exp
    Copy = mybir.ActivationFunctionType.Copy

    from contextlib import ExitStack

    with tile.TileContext(nc) as tc, ExitStack() as ctx:
        const = ctx.enter_context(tc.tile_pool(name="const", bufs=1))
        bk_sb = const.tile([P, E // P], F32)
        ones64 = const.tile([P, HEAD_DIM], BF16)

        persist = ctx.enter_context(tc.tile_pool(name="persist", bufs=1))
        qT = persist.tile([P, E // P, N], BF16)
        kT = persist.tile([P, E // P, M], BF16)
        # cols 0:64 = V, col 64 = ones (denominator lands on psum row 64)
        vaug = persist.tile([P, MT, HPC, HEAD_DIM + 1], BF16)
        woT = persist.tile([P, E // P, DIM], BF16)
        otn2 = persist.tile([P, E // P, N], BF16)

        bwork = ctx.enter_context(tc.tile_pool(name="bwork", bufs=4))
        maskp = ctx.enter_context(tc.tile_pool(name="maskp", bufs=6))
        rbp = ctx.enter_context(tc.tile_pool(name="rbp", bufs=2))

        exmp = ctx.enter_context(tc.tile_pool(name="exmp", bufs=1))
        # masked exp(scores) parked per m-tile; one buffer reused across
        # head pairs (WAR: stage-2 rewrites a tile only after its PV read)
        exmst = exmp.tile([P, MT, 2, N], BF16)

        mask_tiles = {}

        def load_masks(upto):
            """Issue mask DMAs up to tile index `upto`. Stage-1 masks ride
            the gpsimd SWDGE only (the HWDGE rings are busy with x/ctx and a
            queued mask would block them FIFO-wise); stage-2 reloads
            alternate gpsimd/scalar (both idle by then). The 6-buffer pool
            provides back-pressure against loading too far ahead."""
            while len(mask_tiles) < min(upto, 2 * MT):
                i = len(mask_tiles)
                mt = i % MT
                mk = maskp.tile([P, N], BF16, tag="mk", name="mk")
                ring = nc.gpsimd if (i < MT or i % 2) else nc.scalar
                ring.dma_start(out=mk, in_=maskt_d[mt * P : (mt + 1) * P, :])
                mask_tiles[i] = mk

        def scores_half(spool, sbufs, hp, mt, chn):
            """score pair for head pair hp at (m-tile, n-half); the two
            heads run as concurrent row-tiled matmuls."""
            st = spool.tile([P, 2, 512], F32, tag="st", name="st", bufs=sbufs)
            for hl in range(2):
                erow = slice(hl * HEAD_DIM, (hl + 1) * HEAD_DIM)
                nc.tensor.matmul(
                    st[:, hl, :],
                    lhsT=kT[erow, hp, mt * P : (mt + 1) * P],
                    rhs=qT[erow, hp, chn * 512 : (chn + 1) * 512],
                    start=True,
                    stop=True,
                )
            return st

        def expmask_half(st, mt, chn, mk):
            ex = bwork.tile([P, 2, 512], BF16, tag="ex", name="ex")
            nc.scalar.activation(ex, st, Exp, scale=float(SCALE))
            mks = mk[:, chn * 512 : (chn + 1) * 512]
            mkc = bass.AP(mks.tensor, mks.offset, [mks.ap[0], [0, 2], mks.ap[1]])
            nc.vector.tensor_mul(
                exmst[:, mt, :, chn * 512 : (chn + 1) * 512], ex, mkc
            )

        def emit_pv(ot_ps, hp, mt):
            for hl in range(2):
                h = hp * 2 + hl
                for chn in range(2):
                    nc.tensor.matmul(
                        ot_ps[hl * 2 + chn],
                        lhsT=vaug[:, mt, h, :],
                        rhs=exmst[:, mt, hl, chn * 512 : (chn + 1) * 512],
                        start=(mt == 0),
                        stop=(mt == MT - 1),
                    )

        def norm_recip(ot_ps, rbps):
            """stage the denominator row (PSUM partition 64) through SBUF,
            then a K=1 matmul at tile row-position 64 broadcasts it to PSUM
            rows 0:64. (reciprocal_approx_fast only works on SBUF inputs at
            partition base 0 on HW, so the reciprocal happens after.)"""
            dn1 = rbp.tile([P, 512], BF16, tag="dn1", name="dn1")
            nc.vector.tensor_copy(
                dn1[HEAD_DIM : HEAD_DIM + 1, :],
                ot_ps[HEAD_DIM : HEAD_DIM + 1, :],
            )
            dp = rbps.tile([HEAD_DIM, 512], F32, tag="rp", name="rp")
            nc.tensor.matmul(
                dp,
                lhsT=ones64[HEAD_DIM : HEAD_DIM + 1, :],
                rhs=dn1[HEAD_DIM : HEAD_DIM + 1, :],
                start=True,
                stop=True,
            )
            return dp

        def norm_apply(ot_ps, dp, h, chn):
            """reciprocal of the broadcast denominator (SBUF, base 0), then
            multiply rows 0:64 of the PV psum into the head's otn2 slot
            (odd heads shift partitions via DMA)."""
            et, hl = divmod(h, 2)
            sl = slice(chn * 512, (chn + 1) * 512)
            dd = rbp.tile([HEAD_DIM, 512], F32, tag="dd", name="dd")
            nc.vector.tensor_copy(dd, dp)
            rb = rbp.tile([HEAD_DIM, 512], F32, tag="rb", name="rb")
            nc.vector.reciprocal_approx_fast(out=rb, in_=dd)
            if hl == 0:
                nc.vector.tensor_mul(
                    otn2[:HEAD_DIM, et, sl], ot_ps[:HEAD_DIM, :], rb
                )
            else:
                tmp = rbp.tile([HEAD_DIM, 512], BF16, tag="tmp", name="tmp")
                nc.vector.tensor_mul(tmp, ot_ps[:HEAD_DIM, :], rb)
                # partition shift 0:64 -> 64:128 via SBUF-SBUF DMA
                nc.gpsimd.dma_start(out=otn2[HEAD_DIM:P, et, sl], in_=tmp)

        with tc.tile_pool(name="wctx", bufs=1) as wctx_pool:
            wkT = wctx_pool.tile([P, CT, E], BF16)
            wvT = wctx_pool.tile([P, CT, E], BF16)
            ctxT = wctx_pool.tile([P, CHM, CT, 512], BF16)

            with (
                tc.tile_pool(name="qx", bufs=1) as qx_pool,
                tc.tile_pool(name="ppsA", bufs=1, space="PSUM") as ppsA,
                tc.tile_pool(name="vps", bufs=1, space="PSUM") as vps,
                tc.tile_pool(name="kps", bufs=2, space="PSUM") as kps,
            ):
                wqT = qx_pool.tile([P, CT, E], BF16)
                xT = qx_pool.tile([P, CT, N], BF16)

                # ---------- input DMA: large contiguous transfers, ordered
                # per ring for earliest compute start (critical chain:
                # wq+x -> qproj; wk+ctx chunk0 -> kproj -> first scores) ----
                # sync ring: wq, x j0-3 (j-granular so qproj pipelines with
                # arrival), then ctx chunks 2,3
                csz = CT * 512
                nc.sync.dma_start(out=wqT, in_=wqT_d)
                for j in range(4):
                    nc.sync.dma_start(
                        out=xT[:, j, :], in_=xT_d[:, j * N : (j + 1) * N]
                    )
                for c in (2, 3):
                    nc.sync.dma_start(
                        out=ctxT[:, c, :, :], in_=ctxT_d[:, c * csz : (c + 1) * csz]
                    )
                # gpsimd SWDGE: bias, wk, x j4-7, wv, then masks (in-loop)
                nc.gpsimd.dma_start(out=bk_sb, in_=bk_d.rearrange("(t p) -> p t", p=P))
                nc.gpsimd.dma_start(out=wkT, in_=wkT_d)
                for j in range(4, CT):
                    nc.gpsimd.dma_start(
                        out=xT[:, j, :], in_=xT_d[:, j * N : (j + 1) * N]
                    )
                nc.gpsimd.dma_start(out=wvT, in_=wvT_d)
                # scalar ring: ctx chunks 0,1, wo
                for c in (0, 1):
                    nc.scalar.dma_start(
                        out=ctxT[:, c, :, :], in_=ctxT_d[:, c * csz : (c + 1) * csz]
                    )
                nc.scalar.dma_start(out=woT, in_=woT_d)

                def emit_qproj(et, chn):
                    pq = ppsA.tile([P, 512], F32, tag="pq", name="pq")
                    for j in range(CT):
                        nc.tensor.matmul(
                            pq,
                            lhsT=wqT[:, j, et * P : (et + 1) * P],
                            rhs=xT[:, j, chn * 512 : (chn + 1) * 512],
                            start=(j == 0),
                            stop=(j == CT - 1),
                        )
                    nc.vector.tensor_copy(
                        qT[:, et, chn * 512 : (chn + 1) * 512], pq
                    )

                def emit_kproj(et, chm):
                    pk = kps.tile([P, 512], F32, tag="pk", name="pk")
                    for j in range(CT):
                        nc.tensor.matmul(
                            pk,
                            lhsT=wkT[:, j, et * P : (et + 1) * P],
                            rhs=ctxT[:, chm, j, :],
                            start=(j == 0),
                            stop=(j == CT - 1),
                        )
                    nc.vector.tensor_scalar_add(
                        kT[:, et, chm * 512 : (chm + 1) * 512],
                        pk,
                        bk_sb[:, et : et + 1],
                    )

                def emit_vproj(mt):
                    pv = vps.tile([P, HPC, HEAD_DIM], F32, tag="pv", name="pv")
                    for j in range(CT):
                        nc.tensor.matmul(
                            pv,
                            lhsT=ctxT[:, mt // 4, j, (mt % 4) * P : (mt % 4 + 1) * P],
                            rhs=wvT[:, j, :],
                            start=(j == 0),
                            stop=(j == CT - 1),
                        )
                    nc.vector.tensor_copy(vaug[:, mt, :, :HEAD_DIM], pv)

                # ---------- prologue: all of qproj (arrival-paced by the
                # j-granular x tiles), then the first kproj ----------
                # ones column only; v evictions fill cols 0:64
                nc.vector.memset(vaug[:, :, :, HEAD_DIM : HEAD_DIM + 1], 1.0)
                nc.vector.memset(ones64, 1.0)
                emit_qproj(0, 0)
                emit_qproj(0, 1)
                emit_qproj(1, 0)
                emit_qproj(1, 1)
                emit_kproj(0, 0)

                # ---------- stage 1: scores heads 0,1 + projections ----------
                # kproj leftovers keyed to ctx chunk arrival (chm1 on the
                # scalar ring ~t26, chm2/chm3 on the sync ring ~t30/t40)
                s1_extras = {
                    3: lambda: emit_kproj(0, 1),
                    6: lambda: emit_kproj(0, 2),
                    11: lambda: emit_kproj(0, 3),
                }
                with tc.tile_pool(name="sps1", bufs=1, space="PSUM") as sps1:
                    for mt in range(MT):
                        load_masks(mt + 4)
                        mk = mask_tiles[mt]
                        st0 = scores_half(sps1, 2, 0, mt, 0)
                        st1 = scores_half(sps1, 2, 0, mt, 1)
                        expmask_half(st0, mt, 0, mk)
                        expmask_half(st1, mt, 1, mk)
                        if mt >= 2:
                            emit_vproj(mt - 2)
                        if mt in s1_extras:
                            s1_extras[mt]()
                    emit_vproj(MT - 2)
                    emit_vproj(MT - 1)

            # ---------- stage 2: PV heads 0,1 + scores heads 2,3 ----------
            s2_kproj = {0: 0, 2: 1, 5: 2, 8: 3}
            with (
                tc.tile_pool(name="ops0", bufs=1, space="PSUM") as ops0,
                tc.tile_pool(name="sps2", bufs=1, space="PSUM") as sps2,
                tc.tile_pool(name="kps2", bufs=1, space="PSUM") as kps2,
                tc.tile_pool(name="rbps", bufs=1, space="PSUM") as rbps,
            ):
                def emit_kproj2(chm):
                    pk = kps2.tile([P, 512], F32, tag="pk2", name="pk2")
                    for j in range(CT):
                        nc.tensor.matmul(
                            pk,
                            lhsT=wkT[:, j, P : 2 * P],
                            rhs=ctxT[:, chm, j, :],
                            start=(j == 0),
                            stop=(j == CT - 1),
                        )
                    nc.vector.tensor_scalar_add(
                        kT[:, 1, chm * 512 : (chm + 1) * 512],
                        pk,
                        bk_sb[:, 1:2],
                    )

                ot_ps0 = [
                    ops0.tile([HEAD_DIM + 1, 512], F32, tag=f"o{i}", name=f"o{i}")
                    for i in range(4)
                ]
                for mt in range(MT):
                    load_masks(MT + mt + 4)
                    mk = mask_tiles[MT + mt]
                    if mt in s2_kproj:
                        emit_kproj2(s2_kproj[mt])
                    st0 = scores_half(sps2, 1, 1, mt, 0)
                    emit_pv(ot_ps0, 0, mt)
                    expmask_half(st0, mt, 0, mk)
                    st1 = scores_half(sps2, 1, 1, mt, 1)
                    expmask_half(st1, mt, 1, mk)
                # normalize heads 0,1 straight out of PSUM
                for hl in range(2):
                    for chn in range(2):
                        ps = ot_ps0[hl * 2 + chn]
                        rp = norm_recip(ps, rbps)
                        norm_apply(ps, rp, hl, chn)

        # ---------- stage 3: PV heads 2,3 (split by n-half) + output ----------
        with (
            tc.tile_pool(name="ops1", bufs=3, space="PSUM") as ops1,
            tc.tile_pool(name="rbps1", bufs=2, space="PSUM") as rbps1,
            tc.tile_pool(name="ypsum", bufs=3, space="PSUM") as ypsum,
            tc.tile_pool(name="ypool", bufs=4) as ypool,
        ):
            # one rotating tag: (h2,c0),(h3,c0),(h2,c1),(h3,c1) share 3 banks
            # (the 4th chain starts once the 1st is normalized out)
            ot_ps1 = {}
            for chn in range(2):
                for hl in range(2):
                    ot_ps1[(hl, chn)] = ops1.tile(
                        [HEAD_DIM + 1, 512], F32, tag="p1", name=f"p{hl}{chn}"
                    )
            rings = [nc.sync, nc.scalar, nc.gpsimd]
            evict_eng = [nc.scalar, nc.vector]

            def outproj_quarter(c):
                for nb in range(4 * c, 4 * c + 4):
                    for oc in range(OC):
                        i = nb * OC + oc
                        yp = ypsum.tile([P, 512], F32, tag="yp", name="yp")
                        for et in range(E // P):
                            nc.tensor.matmul(
                                yp,
                                lhsT=otn2[:, et, nb * P : (nb + 1) * P],
                                rhs=woT[:, et, oc * 512 : (oc + 1) * 512],
                                start=(et == 0),
                                stop=(et == E // P - 1),
                            )
                        ys = ypool.tile([P, 512], BF16, tag="ys", name="ys")
                        eng = evict_eng[i % 2]
                        if eng is nc.scalar:
                            nc.scalar.activation(ys, yp, Copy)
                        else:
                            nc.vector.tensor_copy(ys, yp)
                        rings[i % 3].dma_start(out=y_d[nb, oc], in_=ys)

            def pv_chain(hl, chn):
                for mt in range(MT):
                    nc.tensor.matmul(
                        ot_ps1[(hl, chn)],
                        lhsT=vaug[:, mt, 2 + hl, :],
                        rhs=exmst[:, mt, hl, chn * 512 : (chn + 1) * 512],
                        start=(mt == 0),
                        stop=(mt == MT - 1),
                    )

            # n-half 0: PV both heads, then normalize (overlaps n-half 1 PV)
            pv_chain(0, 0)
            pv_chain(1, 0)
            for hl in range(2):
                ps = ot_ps1[(hl, 0)]
                rp = norm_recip(ps, rbps1)
                norm_apply(ps, rp, 2 + hl, 0)
            # n-half 1: PV both heads
            pv_chain(0, 1)
            pv_chain(1, 1)
            # reciprocal first (DVE) so outproj q0 keeps the PE busy while
            # the rest of the c1 normalization completes
            rps = []
            for hl in range(2):
                rps.append(norm_recip(ot_ps1[(hl, 1)], rbps1))
            outproj_quarter(0)
            for hl in range(2):
                norm_apply(ot_ps1[(hl, 1)], rps[hl], 2 + hl, 1)
            outproj_quarter(1)

    nc.compile()
    return nc


_NC_CACHE = []


def _get_nc():
    if not _NC_CACHE:
        _NC_CACHE.append(build_program())
    return _NC_CACHE[0]


def _pack(a, p=P):
    """[(j p), f] -> [p, j*f] contiguous (SBUF tile byte layout)."""
    j = a.shape[0] // p
    return np.ascontiguousarray(
        a.reshape(j, p, a.shape[1]).transpose(1, 0, 2).reshape(p, -1)
    )


def make_in_maps(x, context, mask, Wq, Wkv, b_kv, Wo):
    bf = ml_dtypes.bfloat16
    x = np.asarray(x, dtype=np.float32)
    context = np.asarray(context, dtype=np.float32)
    mask = np.asarray(mask)
    Wq = np.asarray(Wq, dtype=np.float32)
    Wkv = np.asarray(Wkv, dtype=np.float32)
    b_kv = np.asarray(b_kv, dtype=np.float32)
    Wo = np.asarray(Wo, dtype=np.float32)

    in_maps = []
    for b in range(B):
        xtb = _pack(x[b].T.astype(bf))
        # ctx swizzled chunk-major: [p, chm, j, 512]
        ctb = np.ascontiguousarray(
            context[b]
            .T.astype(bf)
            .reshape(CT, P, CHM, 512)
            .transpose(1, 2, 0, 3)
            .reshape(P, -1)
        )
        mtb = np.ascontiguousarray(mask[b].T).astype(bf)
        for g in range(NUM_HEADS // HPC):
            sl = slice(E * g, E * (g + 1))
            in_maps.append(
                {
                    "xT": xtb,
                    "ctxT": ctb,
                    "maskt": mtb,
                    "wqT": _pack(np.ascontiguousarray(Wq[sl].T).astype(bf)),
                    "wkT": _pack(np.ascontiguousarray(Wkv[sl].T).astype(bf)),
                    "wvT": _pack(
                        np.ascontiguousarray(
                            Wkv[DIM + E * g : DIM + E * (g + 1)].T
                        ).astype(bf)
                    ),
                    "woT": _pack(np.ascontiguousarray(Wo[:, sl].T).astype(bf)),
                    "bk": np.ascontiguousarray(b_kv[sl]),
                }
            )
    return in_maps


def combine_outputs(ys, b_kv, Wo):
    """ys: list of 8 per-core partial outputs [NB, OC, P, 512] (packed),
    core order (b, g)."""
    b_v = np.asarray(b_kv, dtype=np.float32)[DIM:]
    ybias = np.asarray(Wo, dtype=np.float32) @ b_v  # [DIM]
    out = np.empty((B, N, DIM), dtype=np.float32)
    G = NUM_HEADS // HPC
    for b in range(B):
        acc = np.asarray(ys[G * b], dtype=np.float32)
        for g in range(1, G):
            acc = acc + np.asarray(ys[G * b + g], dtype=np.float32)
        out[b] = acc.transpose(0, 2, 1, 3).reshape(N, DIM) + ybias[None, :]
    return out


def kernel(x, context, mask, Wq, Wkv, b_kv, Wo):
    nc = _get_nc()
    in_maps = make_in_maps(x, context, mask, Wq, Wkv, b_kv, Wo)
    res = run_bass_kernel_spmd(nc, in_maps, core_ids=list(range(8)))
    ys = [m["y"] for m in res.results]
    return combine_outputs(ys, b_kv, Wo)
